# revision 6
# baseline (speedup 1.0000x reference)
"""Trainium2 Bass kernel for nn_Disc_53515292508892 (ragged_sequence).

Computes: src-GRU (H=1024) over ragged [128,64] token batch -> final hidden,
tgt-GRU seeded with it, then a 2-layer head -> logits [64, 2].
(The reference's ref-encoder outputs are computed then deleted -- dead code --
so they are skipped.)

Sharding: data-parallel over batch, B=64 -> 8 sequences per NeuronCore,
GRU weights replicated, no inter-core communication.

v2 design (per-core, fp16 compute, fp32 PSUM):
  - hidden state kept twice:
      h_str  [128, 256] : partition 32j+b (j = col-group, b = batch lane),
                          free = hidden unit within group (256 each)
      hT     [128, 256] : transposed (hidden-on-partition) = matmul lhsT
  - Whh reordered so col-group j holds (r_j | (1-z)_j | n_j) gate columns;
    recurrent matmul runs 4-way col-tiled via tile_position=(0,32j).
  - NO xW precompute phase: the input projection x@Wih^T(+biases) is done
    per step by extra matmuls that ACCUMULATE into the same PSUM banks the
    h-matmuls use.  Each bank's accumulation group is opened by a K=1
    ones-row matmul carrying the biases.  This removes three DVE adds from
    the serial gate chain, removes the per-step xw DMA, and keeps the PE
    busy during the gate window (HAM clock throttling -- see trace notes).
  - h-matmuls ordered r -> n -> z with separate PSUM tiles, so sigmoid(r)
    and the n-path overlap the z-column streaming.
  - hT is updated by transposing the masked delta e = (n-h)*m*z' (PE) and
    adding to hT (DVE) -- transpose starts ~0.3us earlier than transposing
    h_new would allow; h_new = e + h runs in parallel.

Perf ledger (HW exec, NTFF-profiled):
  2.103 ms  v1 baseline (xw precompute -> DRAM -> per-step DMA + DVE adds;
            per-step serial chain ~4 us; PE throttles to 1.2 GHz on
            alternating steps due to ~4.1 us idle gate windows)
  1.581 ms  v2 xw folded into PSUM via per-step x-matmuls + bias ones-rows;
            r->n->z ordering; hT += T(e) delta update.  Trace showed the
            new wall: EVERY PE instruction's sem update drains at ~34ns
            through the PE's update port, and consumers wait behind that
            queue -- 158 PE instrs/step paced the step at ~6.2us.
  v3: this file -- merged r+z bank (N=512 x-matmuls, 138 -> instrs),
      xn staged to SBUF on idle ACT, split hT halves + evens-first k
      (predicted ~5.2 us/step -> ~1.35 ms)
"""

import sys
import functools

sys.path.insert(0, "/opt/trn_rl_repo")

import numpy as np
import concourse.mybir as mybir
from concourse import bacc, tile
from concourse.bass_utils import run_bass_kernel_spmd

f16 = mybir.dt.float16
f32 = mybir.dt.float32
AO = mybir.AluOpType
AF = mybir.ActivationFunctionType

V, D, H = 32000, 512, 1024
T = 128          # steps per GRU (T_SRC = T_TGT = 128)
BL = 8           # batch per core
NCORES = 8
NG = 4           # col-tile groups
GW = 768         # gate columns per group (256 r | 256 z' | 256 n)
KT = H // 128    # 8 k-tiles over hidden
KD = D // 128    # 4 k-tiles over embedding dim


# ----------------------------------------------------------------------------
# host-side weight/layout prep
# ----------------------------------------------------------------------------

def _gate_perm():
    """perm[g_r] = original Whh/Wih row for reordered gate column g_r;
    sign[g_r] = -1 for z columns (z' = 1 - z = sigmoid(-pre_z)); is_n mask."""
    g = np.arange(3072)
    j = g // GW
    u = g % GW
    us = u // 128
    p = u % 128
    within = us * 128 + p  # == u
    row = np.where(
        us < 2,
        256 * j + within,
        np.where(us < 4, 1024 + 256 * j + (within - 256),
                 2048 + 256 * j + (within - 512)),
    )
    sign = np.where((us >= 2) & (us < 4), -1.0, 1.0).astype(np.float32)
    is_n = us >= 4
    return row, sign, is_n


def _prep_shared(inputs, n_steps):
    """Core-independent tensors (weights, bias rows, identity)."""
    row, sign, is_n = _gate_perm()
    out = {}
    for g, wih, whh, bih, bhh in (
        ("src", inputs["src_Wih"], inputs["src_Whh"], inputs["src_bih"], inputs["src_bhh"]),
        ("tgt", inputs["tgt_Wih"], inputs["tgt_Whh"], inputs["tgt_bih"], inputs["tgt_bhh"]),
    ):
        whh_a = (whh[row] * sign[:, None]).T.astype(np.float16)          # [1024, 3072]
        out[f"whh_{g}"] = np.ascontiguousarray(whh_a.reshape(KT, 128, 3072))
        wih_a = (wih[row] * sign[:, None]).T.astype(np.float16)          # [512, 3072]
        out[f"wih_{g}"] = np.ascontiguousarray(wih_a.reshape(KD, 128, 3072))
        # bias row, permuted order: r/z cols get sign*(bih+bhh); n cols bih
        bias_vec = sign * bih[row] + sign * np.where(is_n, 0.0, bhh[row])
        out[f"brow_{g}"] = bias_vec.astype(np.float16).reshape(1, 3072)
        # pnx bank init row: per group j, [bhh_n_j (256) | bih_n_j (256)]
        bnx = np.zeros((1, NG * 512), np.float16)
        for j in range(NG):
            bnx[0, 512 * j:512 * j + 256] = bhh[2048 + 256 * j: 2048 + 256 * (j + 1)]
            bnx[0, 512 * j + 256:512 * (j + 1)] = bias_vec[GW * j + 512: GW * (j + 1)]
        out[f"bnx_{g}"] = bnx
    p1 = inputs["p1_W"].T.reshape(KT, 128, 64).transpose(1, 0, 2).reshape(128, KT * 64)
    out["p1T"] = p1.astype(np.float16)
    out["p1b"] = np.broadcast_to(inputs["p1_b"].astype(np.float16), (128, 64)).copy()
    out["p2T"] = inputs["p2_W"].T.astype(np.float16)                      # [64, 2]
    out["p2b"] = np.broadcast_to(inputs["p2_b"].astype(np.float32), (128, 2)).copy()
    out["ident"] = np.eye(128, dtype=np.float16)
    return out


def _prep_core(inputs, emb16, core, n_steps):
    """Per-core tensors: gathered/transposed token embeddings and masks."""
    sl = slice(BL * core, BL * (core + 1))
    out = {}
    for g, ids_key in (("src", "src"), ("tgt", "tgt")):
        ids = np.asarray(inputs[ids_key])[:n_steps, sl]                   # [T, 8]
        x = emb16[ids]                                                    # [T, 8, 512]
        out[f"xT_{g}"] = np.ascontiguousarray(
            x.transpose(2, 0, 1).reshape(KD, 128, n_steps * BL))
    masks = np.zeros((128, 2 * n_steps), np.float32)
    for gi, len_key in enumerate(("src_lengths", "tgt_lengths")):
        ln = np.asarray(inputs[len_key])[sl]                              # [8]
        t = np.arange(n_steps)
        m = (t[None, :] < ln[:, None]).astype(np.float32)                 # [8, T]
        for j in range(NG):
            masks[32 * j:32 * j + 8, gi * n_steps:(gi + 1) * n_steps] = m
    out["masks"] = masks
    return out


# ----------------------------------------------------------------------------
# device program
# ----------------------------------------------------------------------------

def build_program(n_steps=T, debug=False):
    nc = bacc.Bacc("TRN2", target_bir_lowering=False, debug=False,
                   num_devices=NCORES)
    TB = n_steps * BL
    S = 2 * n_steps

    dp = nc.declare_dram_parameter
    d_xT = {g: dp(f"xT_{g}", [KD, 128, TB], f16, isOutput=False) for g in ("src", "tgt")}
    d_whh = {g: dp(f"whh_{g}", [KT, 128, 3072], f16, isOutput=False) for g in ("src", "tgt")}
    d_wih = {g: dp(f"wih_{g}", [KD, 128, 3072], f16, isOutput=False) for g in ("src", "tgt")}
    d_brow = {g: dp(f"brow_{g}", [1, 3072], f16, isOutput=False) for g in ("src", "tgt")}
    d_bnx = {g: dp(f"bnx_{g}", [1, NG * 512], f16, isOutput=False) for g in ("src", "tgt")}
    d_masks = dp("masks", [128, 2 * n_steps], f32, isOutput=False)
    d_p1T = dp("p1T", [128, KT * 64], f16, isOutput=False)
    d_p1b = dp("p1b", [128, 64], f16, isOutput=False)
    d_p2T = dp("p2T", [64, 2], f16, isOutput=False)
    d_p2b = dp("p2b", [128, 2], f32, isOutput=False)
    d_ident = dp("ident", [128, 128], f16, isOutput=False)
    d_logits = dp("logits", [BL, 2], f32, isOutput=True)
    if debug:
        d_dbg_h = dp("dbg_h", [128, 256], f16, isOutput=True)

    with tile.TileContext(nc) as tc:
        with tc.tile_pool(name="const", bufs=1) as cpool, \
             tc.tile_pool(name="work", bufs=2) as wpool, \
             tc.tile_pool(name="psum", bufs=2, space="PSUM") as psum:

            # ---- resident constants -------------------------------------
            whh_sb, wih_sb, xT_sb, brow_sb, bnx_sb = {}, {}, {}, {}, {}
            for g in ("src", "tgt"):
                whh_sb[g] = cpool.tile([128, KT * 3072], f16, tag=f"whh_{g}", name=f"whh_{g}")
                for k in range(KT):
                    nc.sync.dma_start(whh_sb[g][:, 3072 * k:3072 * (k + 1)], d_whh[g][k])
                wih_sb[g] = cpool.tile([128, KD * 3072], f16, tag=f"wih_{g}", name=f"wih_{g}")
                for k in range(KD):
                    nc.sync.dma_start(wih_sb[g][:, 3072 * k:3072 * (k + 1)], d_wih[g][k])
                xT_sb[g] = cpool.tile([128, KD * TB], f16, tag=f"xT_{g}", name=f"xT_{g}")
                for k in range(KD):
                    nc.sync.dma_start(xT_sb[g][:, TB * k:TB * (k + 1)], d_xT[g][k])
                brow_sb[g] = cpool.tile([1, 3072], f16, tag=f"brow_{g}", name=f"brow_{g}")
                nc.sync.dma_start(brow_sb[g][:], d_brow[g][:])
                bnx_sb[g] = cpool.tile([1, NG * 512], f16, tag=f"bnx_{g}", name=f"bnx_{g}")
                nc.sync.dma_start(bnx_sb[g][:], d_bnx[g][:])
            masks_sb = cpool.tile([128, 2 * n_steps], f32, tag="masks")
            nc.sync.dma_start(masks_sb[:], d_masks[:])
            ident_sb = cpool.tile([128, 128], f16, tag="ident")
            nc.sync.dma_start(ident_sb[:], d_ident[:])
            p1T_sb = cpool.tile([128, KT * 64], f16, tag="p1T")
            nc.sync.dma_start(p1T_sb[:], d_p1T[:])
            p1b_sb = cpool.tile([128, 64], f16, tag="p1b")
            nc.sync.dma_start(p1b_sb[:], d_p1b[:])
            p2T_sb = cpool.tile([64, 2], f16, tag="p2T")
            nc.sync.dma_start(p2T_sb[:], d_p2T[:])
            p2b_sb = cpool.tile([128, 2], f32, tag="p2b")
            nc.sync.dma_start(p2b_sb[:], d_p2b[:])
            ones_sb = cpool.tile([1, BL], f16, tag="ones")
            nc.vector.memset(ones_sb[:], 1.0)

            def x_phase(step):
                """Open this step's PSUM accumulation groups with bias rows,
                then accumulate the input projection x_t @ Wih^T.  All of
                this is h-independent, so it runs inside the previous step's
                gate window and keeps the PE clocked up.  r+z share one bank
                (prz) so their x-matmuls merge to N=512 -- every PE
                instruction costs ~34ns on the semaphore-update pipeline,
                which paces the whole step, so instruction count matters as
                much as stream time."""
                g = "src" if step < n_steps else "tgt"
                t = step % n_steps
                has_h = step > 0
                prz = psum.tile([128, 512], f32, tag="prz", name="prz")
                pnx = psum.tile([128, 512], f32, tag="pnx", name="pnx")
                # bank-group openers (start=True) carrying the biases
                for j in range(NG):
                    nc.tensor.matmul(
                        pnx[32 * j:32 * j + BL, :], ones_sb[0:1, :],
                        bnx_sb[g][0:1, 512 * j:512 * (j + 1)],
                        start=True, stop=False, tile_position=(0, 32 * j))
                for j in range(NG):
                    nc.tensor.matmul(
                        prz[32 * j:32 * j + BL, :], ones_sb[0:1, :],
                        brow_sb[g][0:1, GW * j: GW * j + 512],
                        start=True, stop=False, tile_position=(0, 32 * j))
                # x@Wih^T accumulation (r+z cols merged -> prz; n -> pnx hi)
                for kd in range(KD):
                    lhsT = xT_sb[g][:, TB * kd + BL * t: TB * kd + BL * (t + 1)]
                    last = (kd == KD - 1) and not has_h
                    for j in range(NG):
                        nc.tensor.matmul(
                            prz[32 * j:32 * j + BL, :], lhsT,
                            wih_sb[g][:, 3072 * kd + GW * j:
                                      3072 * kd + GW * j + 512],
                            start=False, stop=last, tile_position=(0, 32 * j))
                    for j in range(NG):
                        nc.tensor.matmul(
                            pnx[32 * j:32 * j + BL, 256:512], lhsT,
                            wih_sb[g][:, 3072 * kd + GW * j + 512:
                                      3072 * kd + GW * (j + 1)],
                            start=False, stop=last, tile_position=(0, 32 * j))
                return prz, pnx

            # r-gate k-order: even k (hT block c=0) first, so the next
            # step's first matmuls can start after only half the hT update
            KORD = [0, 2, 4, 6, 1, 3, 5, 7]

            def h_phase(step, prz, pnx, hT):
                """Recurrent matmuls, r -> n -> z order (r/z split keeps
                sigmoid(r) off the z-stream's tail; n before z so the
                n-path overlaps z streaming)."""
                g = "src" if step < n_steps else "tgt"
                # stop closes each bank's accumulation group on its final
                # writer: pnx on h-n k-last, prz on h-z k-last
                for c0, dst, cw, kord, stops in ((0, prz, 0, KORD, False),
                                                 (512, pnx, 0, range(KT), True),
                                                 (256, prz, 256, range(KT), True)):
                    for ki, k in enumerate(kord):
                        coff = 128 * (k % 2) + 32 * (k // 2)
                        lhsT = hT[:, coff:coff + BL]
                        for j in range(NG):
                            nc.tensor.matmul(
                                dst[32 * j:32 * j + BL, cw:cw + 256], lhsT,
                                whh_sb[g][:, 3072 * k + GW * j + c0:
                                          3072 * k + GW * j + c0 + 256],
                                start=False,
                                stop=(stops and ki == KT - 1),
                                tile_position=(0, 32 * j))

            # ---- recurrence ---------------------------------------------
            h_str = wpool.tile([128, 256], f16, tag="h_str", name="h0")
            hT = wpool.tile([128, 256], f16, tag="hT", name="hT0")
            nc.vector.memset(h_str[:], 0.0)
            nc.vector.memset(hT[:], 0.0)

            xts = x_phase(0)
            for step in range(S):
                t = step % n_steps
                mcol = t if step < n_steps else n_steps + t
                prz, pnx = xts

                if step > 0:
                    h_phase(step, prz, pnx, hT)
                if step + 1 < S:
                    xts = x_phase(step + 1)

                # stage xn out of PSUM on the (idle) ACT engine so the
                # n-path add reads SBUF (saves PSUM access latency on DVE)
                xn_sb = wpool.tile([128, 256], f16, tag="xn_sb")
                nc.scalar.activation(xn_sb[:], pnx[:, 256:512], AF.Copy)

                # gates (strip view [128, 256]; only partitions 32j+b<8 valid)
                r_t = wpool.tile([128, 256], f16, tag="r_t")
                nc.scalar.activation(r_t[:], prz[:, 0:256], AF.Sigmoid)
                tn2 = wpool.tile([128, 256], f16, tag="tn2")
                nc.vector.tensor_mul(tn2[:], pnx[:, 0:256], r_t[:])
                sn = wpool.tile([128, 256], f16, tag="sn")
                nc.vector.tensor_add(sn[:], tn2[:], xn_sb[:])
                n_t = wpool.tile([128, 256], f16, tag="n_t")
                nc.scalar.activation(n_t[:], sn[:], AF.Tanh)
                z_t = wpool.tile([128, 256], f16, tag="z_t")
                nc.scalar.activation(z_t[:], prz[:, 256:512], AF.Sigmoid)

                d_t = wpool.tile([128, 256], f16, tag="d_t")
                nc.vector.tensor_sub(d_t[:], n_t[:], h_str[:])
                # e = (n - h) * m * z'   (masked update delta)
                e_t = wpool.tile([128, 256], f16, tag="e_t")
                nc.vector.scalar_tensor_tensor(
                    e_t[:], d_t[:], masks_sb[:, mcol:mcol + 1], z_t[:],
                    AO.mult, AO.mult)

                # hT += T(e)  (transpose the delta; linearity of transpose).
                # hT update split into halves so next step's even-k matmuls
                # (which read only block c=0) start after the first half.
                tp = psum.tile([128, 256], f16, tag="tp", name="tp", bufs=1)
                for c in range(2):
                    nc.tensor.transpose(tp[:, 128 * c:128 * (c + 1)],
                                        e_t[:, 128 * c:128 * (c + 1)],
                                        ident_sb[:])
                hT_new = wpool.tile([128, 256], f16, tag="hT", name="hT_new")
                for c in range(2):
                    nc.vector.tensor_add(hT_new[:, 128 * c:128 * (c + 1)],
                                         tp[:, 128 * c:128 * (c + 1)],
                                         hT[:, 128 * c:128 * (c + 1)])
                h_new = wpool.tile([128, 256], f16, tag="h_str", name="h_new")
                nc.vector.tensor_add(h_new[:], e_t[:], h_str[:])

                h_str, hT = h_new, hT_new

            # ---- head ----------------------------------------------------
            ph = psum.tile([128, 512], f32, tag="ph", name="ph", bufs=1)
            for k in range(KT):
                coff = 128 * (k % 2) + 32 * (k // 2)
                nc.tensor.matmul(
                    ph[0:BL, 0:64],
                    hT[:, coff:coff + BL],
                    p1T_sb[:, 64 * k:64 * (k + 1)],
                    start=(k == 0), stop=(k == KT - 1),
                )
            t1s = wpool.tile([128, 64], f16, tag="t1s")
            nc.vector.tensor_add(t1s[0:BL, :], ph[0:BL, 0:64], p1b_sb[0:BL, :])
            t1 = wpool.tile([128, 64], f16, tag="t1")
            nc.scalar.activation(t1[0:BL, :], t1s[0:BL, :], AF.Tanh)

            pt1 = psum.tile([128, 256], f16, tag="tp", name="pt1", bufs=1)
            nc.tensor.transpose(pt1[0:64, 0:BL], t1[0:BL, 0:64], ident_sb[0:BL, 0:BL])
            t1T = wpool.tile([64, BL], f16, tag="t1T")
            nc.vector.tensor_copy(t1T[:], pt1[0:64, 0:BL])

            pl = psum.tile([128, 512], f32, tag="ph", name="pl", bufs=1)
            nc.tensor.matmul(pl[0:BL, 0:2], t1T[:], p2T_sb[:], start=True, stop=True)
            lg = wpool.tile([128, 2], f32, tag="lg")
            nc.vector.tensor_add(lg[0:BL, :], pl[0:BL, 0:2], p2b_sb[0:BL, :])
            nc.sync.dma_start(d_logits[:], lg[0:BL, :])

            if debug:
                nc.sync.dma_start(d_dbg_h[:], h_str[:])

    nc.compile()
    return nc


# ----------------------------------------------------------------------------
# entry point
# ----------------------------------------------------------------------------

@functools.lru_cache(maxsize=2)
def _cached_program(n_steps, debug):
    return build_program(n_steps, debug)


def run(inputs, n_steps=T, debug=False, trace=False):
    inputs = {k: np.asarray(v) for k, v in inputs.items()}
    nc = _cached_program(n_steps, debug)
    shared = _prep_shared(inputs, n_steps)
    emb16 = np.asarray(inputs["emb"]).astype(np.float16)
    in_maps = []
    for c in range(NCORES):
        m = dict(shared)
        m.update(_prep_core(inputs, emb16, c, n_steps))
        in_maps.append(m)
    res = run_bass_kernel_spmd(nc, in_maps, list(range(NCORES)), trace=trace)
    logits = np.concatenate([res.results[c]["logits"] for c in range(NCORES)], axis=0)
    return logits, res


def kernel(**inputs) -> np.ndarray:
    logits, _ = run(inputs)
    return logits.astype(np.float32)
